# revision 39
# baseline (speedup 1.0000x reference)
"""nn_GateModLinear on 8 trn2 NeuronCores.

z[b,:] = gW[b,:] * sum_m pW[b,m] * (Ws[m] @ x[b]) + gb[b,:] * (pb @ bs)[b,:]
out = ELU(LayerNorm(z))

Sharding: data-parallel over batch (512 rows/core), Ws replicated.
Per core: fold pW into x per expert (host-precomputed xs[m] = pW[:,m]*x,
0.05% of FLOPs), then accumulate all (m, j) into PSUM on the PE:
  Wx[b,i] = sum_{m,j} xs[m,b,j] * Ws[m,i,j]
bf16 matmuls (rel-err budget 2e-2), fp32 PSUM/LayerNorm.
"""

import numpy as np
import ml_dtypes

B, M, DI, DO = 4096, 8, 2048, 2048
NCORES = 8
BS = B // NCORES  # 512 batch rows per core
LN_EPS = 1e-5
P = 128
JC = DI // P      # 16 contraction chunks of 128
BC = BS // P      # 4 batch chunks of 128
NIH = 2           # output-dim halves
IH = DO // NIH    # 1024
NQ = IH // 512    # 2 psum tiles of 512 per half

BF16 = ml_dtypes.bfloat16

_cache = {}


def _build():
    from contextlib import ExitStack
    import concourse.bacc as bacc
    import concourse.tile as tile
    from concourse import mybir

    f32 = mybir.dt.float32
    bf16 = mybir.dt.bfloat16
    i32 = mybir.dt.int32
    AF = mybir.ActivationFunctionType
    ALU = mybir.AluOpType

    nc = bacc.Bacc("TRN2", target_bir_lowering=False, debug=False, num_devices=1)
    xs_d = nc.dram_tensor("xs", [M, DI, BS], bf16, kind="ExternalInput")
    ws_d = nc.dram_tensor("wsT", [M, DI, DO], bf16, kind="ExternalInput")
    pb_d = nc.dram_tensor("pbT", [M, BS], f32, kind="ExternalInput")
    bs_d = nc.dram_tensor("bs", [M, DO], f32, kind="ExternalInput")
    gw_d = nc.dram_tensor("gw", [BS, DO], bf16, kind="ExternalInput")
    gb_d = nc.dram_tensor("gb", [BS, DO], bf16, kind="ExternalInput")
    out_d = nc.dram_tensor("out", [BS, DO], bf16, kind="ExternalOutput")

    with ExitStack() as ctx:
        tc = ctx.enter_context(tile.TileContext(nc))
        singles = ctx.enter_context(tc.tile_pool(name="singles", bufs=1))
        ws_pool = ctx.enter_context(tc.tile_pool(name="ws", bufs=8))
        xs_pool = ctx.enter_context(tc.tile_pool(name="xs", bufs=5))
        e_pool = ctx.enter_context(tc.tile_pool(name="elu", bufs=2))
        sm_pool = ctx.enter_context(tc.tile_pool(name="small", bufs=4))
        ps_pool = ctx.enter_context(tc.tile_pool(name="ps", bufs=8, space="PSUM"))

        phases = [(ih, m) for ih in range(NIH) for m in range(M)]

        WCH = 4           # ws jc-chunks per phase
        WJ = JC // WCH    # 4 jc per ws chunk
        XCH = 2           # xs jc-chunks per phase
        XJ = JC // XCH    # 8 jc per xs chunk

        def load(idx):
            ih, m = phases[idx]
            xsrc = xs_d.ap()[m].rearrange("(jc jp) b -> jp jc b", jp=P)
            xts = []
            for h in range(XCH):
                t = xs_pool.tile([P, XJ, BS], bf16, tag="xs",
                                 name=f"xs_{ih}_{m}_{h}")
                nc.gpsimd.dma_start(
                    out=t, in_=xsrc[:, h * XJ:(h + 1) * XJ, :]
                )
                xts.append(t)
            wsrc = ws_d.ap()[m].rearrange("(jc jp) i -> jp jc i", jp=P)
            wst = []
            for h in range(WCH):
                t = ws_pool.tile([P, WJ, IH], bf16, tag="ws",
                                 name=f"ws_{ih}_{m}_{h}")
                nc.sync.dma_start(
                    out=t,
                    in_=wsrc[:, h * WJ:(h + 1) * WJ,
                             ih * IH:(ih + 1) * IH],
                )
                wst.append(t)
            return xts, wst

        # ---- PE warm-up: full-K dummy matmuls with no DMA deps keep the
        # PE array-utilization monitor busy so the clock ramps to 2.4 GHz
        # before the real stream starts ----
        wl = singles.tile([P, P], bf16)
        nc.vector.memset(wl, 1.0)
        wr = singles.tile([P, 512], bf16)
        nc.vector.memset(wr, 0.5)
        wp = ps_pool.tile([P, 512], f32, tag="acc", name="warm")
        for _ in range(8):
            nc.tensor.matmul(wp, wl, wr, start=True, stop=True)

        # ---- small loads (cheap, needed by bias matmuls).
        # K=8 matmuls look idle to the PE activity monitor and drop the
        # clock to 1.2 GHz for ~30us, so pad the bias operands to K=128
        # (zeroed on-chip; only the 8 real rows come over DMA).
        pbT = singles.tile([P, BS], f32)
        nc.gpsimd.memset(pbT, 0.0)
        nc.sync.dma_start(out=pbT[:M, :], in_=pb_d.ap())
        bs_sb = singles.tile([P, DO], f32)
        nc.gpsimd.memset(bs_sb, 0.0)
        nc.sync.dma_start(out=bs_sb[:M, :], in_=bs_d.ap())

        # prefetch phase 0 (xs on gpsimd queue, ws on sync queue)
        pending = load(0)

        gb = singles.tile([P, BC, DO], bf16)
        nc.gpsimd.dma_start(
            out=gb, in_=gb_d.ap().rearrange("(bc p) i -> p bc i", p=P)
        )
        z = singles.tile([P, BC, DO], f32)

        # ---- bias: z = pb @ bs (drained via ACT copy — no gb dependency,
        # so psum slots recycle fast); gb multiply applied below once loaded
        for bc in range(BC):
            for q in range(DO // 512):
                bps = ps_pool.tile([P, 512], f32, tag="acc", name=f"bps_{bc}_{q}")
                nc.tensor.matmul(
                    bps,
                    pbT[:, bc * P:(bc + 1) * P],
                    bs_sb[:, q * 512:(q + 1) * 512],
                    start=True, stop=True,
                )
                if q % 2 == 0:
                    nc.scalar.copy(z[:, bc, q * 512:(q + 1) * 512], bps)
                else:
                    nc.vector.tensor_copy(z[:, bc, q * 512:(q + 1) * 512], bps)
        for bc in range(BC):
            nc.vector.tensor_mul(z[:, bc, :], z[:, bc, :], gb[:, bc, :])

        gw = singles.tile([P, BC, DO], bf16)
        nc.gpsimd.dma_start(
            out=gw, in_=gw_d.ap().rearrange("(bc p) i -> p bc i", p=P)
        )

        out_ap = out_d.ap().rearrange("(bc p) i -> p bc i", p=P)

        def drain(ih, acc, bc):
            for q in range(NQ):
                i0 = ih * IH + q * 512
                nc.vector.tensor_mul(acc[bc][q], acc[bc][q],
                                     gw[:, bc, i0:i0 + 512])
                nc.vector.tensor_add(z[:, bc, i0:i0 + 512],
                                     z[:, bc, i0:i0 + 512], acc[bc][q])

        def epilogue(bc):
            # LayerNorm + ELU + store for batch chunk bc
            row = z[:, bc, :]
            stats = sm_pool.tile([P, DO // 512, 6], f32, tag="stats",
                                 name=f"stats_{bc}")
            for s in range(DO // 512):
                nc.vector.bn_stats(out=stats[:, s, :],
                                   in_=row[:, s * 512:(s + 1) * 512])
            mv = sm_pool.tile([P, 2], f32, tag="mv", name=f"mv_{bc}")
            nc.vector.bn_aggr(out=mv, in_=stats)
            # rstd = 1/sqrt(var+eps) on DVE (bitcast seed + 2 Newton steps)
            # so the ACT engine's function table stays pinned to Exp.
            rstd = sm_pool.tile([P, 1], f32, tag="rstd", name=f"rstd_{bc}")
            ve = sm_pool.tile([P, 1], f32, tag="ve", name=f"ve_{bc}")
            nc.vector.tensor_scalar_add(ve, mv[:, 1:2], LN_EPS)  # v + eps
            vh = sm_pool.tile([P, 1], f32, tag="vh", name=f"vh_{bc}")
            nc.vector.tensor_scalar_mul(vh, ve, 0.5)
            # seed bits: 0x5f3759df - (i >> 1)  ==  ((i>>1) ^ ~0) + 0x5f3759e0
            nc.vector.tensor_scalar(
                rstd.bitcast(i32), ve.bitcast(i32), 1, -1,
                op0=ALU.logical_shift_right, op1=ALU.bitwise_xor)
            nc.vector.tensor_scalar_add(rstd.bitcast(i32), rstd.bitcast(i32),
                                        0x5f3759e0)
            for _ in range(1):  # y *= 1.5 - vh*y*y
                t1 = sm_pool.tile([P, 1], f32, tag="t1", name=f"t1_{bc}")
                nc.vector.tensor_mul(t1, rstd, rstd)
                nc.vector.tensor_mul(t1, t1, vh)
                nc.vector.tensor_scalar(t1, t1, -1.0, -1.5,
                                        op0=ALU.mult, op1=ALU.subtract)
                nc.vector.tensor_mul(rstd, rstd, t1)
            nmu = sm_pool.tile([P, 1], f32, tag="nmu", name=f"nmu_{bc}")
            nc.vector.tensor_scalar_mul(nmu, mv[:, 0:1], -1.0)
            nmr = sm_pool.tile([P, 1], f32, tag="nmr", name=f"nmr_{bc}")
            nc.vector.tensor_mul(nmr, nmu, rstd)
            # per output half: et = exp(y) on ACT || y on DVE, then fuse+store
            for h in range(2):
                hs = slice(h * (DO // 2), (h + 1) * (DO // 2))
                rh = row[:, hs]
                et = e_pool.tile([P, DO // 2], bf16, tag="et", name=f"et_{bc}_{h}")
                nc.scalar.activation(out=et, in_=rh, func=AF.Exp,
                                     bias=nmr, scale=rstd)
                yt = e_pool.tile([P, DO // 2], bf16, tag="yt", name=f"yt_{bc}_{h}")
                nc.vector.tensor_scalar(yt, rh, nmu, rstd,
                                        op0=ALU.add, op1=ALU.mult)
                nc.vector.tensor_scalar(et, et, -1.0, 0.0,
                                        op0=ALU.add, op1=ALU.min)
                ot = e_pool.tile([P, DO // 2], bf16, tag="ot", name=f"ot_{bc}_{h}")
                nc.vector.scalar_tensor_tensor(ot, yt, 0.0, et,
                                               op0=ALU.max, op1=ALU.add)
                eng = nc.sync if bc == BC - 1 else nc.gpsimd
                eng.dma_start(out=out_ap[:, bc, hs], in_=ot)

        # ---- main accumulation ----
        for idx, (ih, m) in enumerate(phases):
            xts, wst = pending
            if idx + 1 < len(phases):
                pending = load(idx + 1)
            if m == 0:
                acc = [[ps_pool.tile([P, 512], f32, tag="acc",
                                     name=f"acc_{ih}_{bc}_{q}")
                        for q in range(NQ)] for bc in range(BC)]
            last = (m == M - 1)
            if last:
                # bc-major so each chunk finishes early and its epilogue
                # overlaps the remaining chunks' matmuls
                for bc in range(BC):
                    for jc in range(JC):
                        xt = xts[jc // XJ]
                        w = wst[jc // WJ]
                        for q in range(NQ):
                            nc.tensor.matmul(
                                acc[bc][q],
                                xt[:, jc % XJ, bc * P:(bc + 1) * P],
                                w[:, jc % WJ, q * 512:(q + 1) * 512],
                                start=False,
                                stop=(jc == JC - 1),
                            )
                    drain(ih, acc, bc)
                    if ih == NIH - 1:
                        epilogue(bc)
            else:
                for jc in range(JC):
                    xt = xts[jc // XJ]
                    w = wst[jc // WJ]
                    for bc in range(BC):
                        for q in range(NQ):
                            nc.tensor.matmul(
                                acc[bc][q],
                                xt[:, jc % XJ, bc * P:(bc + 1) * P],
                                w[:, jc % WJ, q * 512:(q + 1) * 512],
                                start=(m == 0 and jc == 0),
                                stop=False,
                            )

    nc.compile()
    return nc


def _prep_inputs(x, Ws, bs, pW, pb, gW, gb):
    x = np.asarray(x, np.float32)
    pW = np.asarray(pW, np.float32)
    # xs[m, j, b] = pW[b, m] * x[b, j], bf16, per-core column slices
    xT = np.ascontiguousarray(x.T)                        # [DI, B]
    wsT = np.ascontiguousarray(
        np.asarray(Ws, np.float32).transpose(0, 2, 1)
    ).astype(BF16)                                        # [M, DI, DO]
    pbT = np.ascontiguousarray(np.asarray(pb, np.float32).T)  # [M, B]
    bs_pad = np.ascontiguousarray(np.asarray(bs, np.float32))
    gW16 = np.asarray(gW, np.float32).astype(BF16)
    gb16 = np.asarray(gb, np.float32).astype(BF16)
    in_maps = []
    for c in range(NCORES):
        sl = slice(c * BS, (c + 1) * BS)
        xs = (pW[sl].T[:, None, :] * xT[None, :, sl]).astype(BF16)
        in_maps.append({
            "xs": np.ascontiguousarray(xs),               # [M, DI, BS]
            "wsT": wsT,
            "pbT": np.ascontiguousarray(pbT[:, sl]),
            "bs": bs_pad,
            "gw": np.ascontiguousarray(gW16[sl]),
            "gb": np.ascontiguousarray(gb16[sl]),
        })
    return in_maps


def kernel(x, Ws, bs, pW, pb, gW, gb, _trace=False, _tmpdir=None):
    from concourse import bass_utils

    if "nc" not in _cache:
        _cache["nc"] = _build()
    nc = _cache["nc"]
    in_maps = _prep_inputs(x, Ws, bs, pW, pb, gW, gb)
    res = bass_utils.run_bass_kernel_spmd(
        nc, in_maps, core_ids=list(range(NCORES)),
        trace=_trace, tmpdir=_tmpdir,
    )
    _cache["last_result"] = res
    out = np.concatenate([res.results[c]["out"] for c in range(NCORES)], axis=0)
    return np.asarray(out, dtype=np.float32)
